# revision 92
# baseline (speedup 1.0000x reference)
"""BitLinear forward on 8 Trainium2 NeuronCores.

Computes y = x @ (unpack_bits(bp).reshape(OUT, IN) * scale).T for
x[64, 4096] fp32, bp[OUT*IN/8] int32 (8 sign bits per int, MSB-first),
scale[OUT, 1] fp32, OUT=11008, IN=4096.

Strategy (column-parallel / output-feature sharded, no collectives):
  * Each core owns 1376 output rows, padded to 1408 = 11 * 128.
  * Host re-lays bp as uint16 bpT[g, o] (g = in-feature group of 8) and
    x as 2 * x.T with rows permuted to (bit j, group g) order, bf16.
  * Device unpack on DVE: MSB plane via is_ge (casts u16->bf16), LSB
    plane as +-1 bf16 bit patterns via one op ((v << 15) ^ 0xBF80, read
    through a bitcast), the rest as bitwise (v>>s)&1 in uint16 plus a
    dtype-cast copy split across DVE and ScalarE (bitwise TSP ops
    cannot cast). A c0-only sweep over all bits runs while the later
    DMA chunks are still in flight.
  * PE accumulates 96 bf16 matmuls psum[t, o] += xw_jc.T @ plane_jc
    with column tiling: even-c rounds write rows 0:64 of one PSUM bank,
    odd-c rounds rows 64:128 of another, so two matmuls run
    concurrently in the PE array.
  * Epilogue per output chunk (og-major on the last round so it
    overlaps remaining matmuls): PSUM->SBUF copies add the per-token
    -sum(x) correction (w = 2b - 1) as a bias; an fp16 matmul against a
    stacked [I; I] matrix transposes y to [o, t] and sums the parity
    halves; DVE applies the per-output-row scale; output DMA is
    chunked.
  * Host concatenates core outputs and transposes back to [64, OUT].
"""

import numpy as np
import ml_dtypes

OUT, IN, TOKENS = 11008, 4096, 64
NCORES = 8
P = 128
G = IN // 8              # 512 in-feature groups (bytes per output row)
OPC = 1408               # padded output rows per core (11 * 128)
W4 = 4 * OPC             # quad width: all four g-chunks side by side
OUT_PAD = NCORES * OPC   # 11264
KCH = OPC // P           # 11 output chunks of 128 rows per core
OG_SIZES = [512, 512, 384]  # psum free-dim chunking of 1408
OG_STARTS = [0, 512, 1024]
OG_KS = [range(0, 4), range(4, 8), range(8, 11)]  # 128-chunks per og

_CACHE = {}


def _build_bass():
    """Build + compile the per-core Bass kernel (identical on all cores)."""
    from contextlib import ExitStack

    import concourse.bass as bass
    import concourse.mybir as mybir
    import concourse.tile as tile
    from concourse import bacc
    from concourse.masks import make_identity

    nc = bacc.Bacc("TRN2", target_bir_lowering=False, debug=False)

    bpt = nc.dram_tensor("bpt", (G, OPC), mybir.dt.uint16, kind="ExternalInput")
    xt = nc.dram_tensor("xt", (P, 32 * TOKENS), mybir.dt.bfloat16, kind="ExternalInput")
    negsx = nc.dram_tensor("negsx", (P, 1), mybir.dt.float32, kind="ExternalInput")
    scale_t = nc.dram_tensor("scale_t", (P, KCH), mybir.dt.float32, kind="ExternalInput")
    yt = nc.dram_tensor("yt", (P, KCH * TOKENS), mybir.dt.float32, kind="ExternalOutput")

    with tile.TileContext(nc) as tc, ExitStack() as ctx:
        consts = ctx.enter_context(tc.tile_pool(name="consts", bufs=1))
        plane_pool = ctx.enter_context(tc.tile_pool(name="planes", bufs=5))
        upool = ctx.enter_context(tc.tile_pool(name="uplanes", bufs=4))
        out_pool = ctx.enter_context(tc.tile_pool(name="outs", bufs=1))
        psum_y = ctx.enter_context(tc.tile_pool(name="psum_y", bufs=1, space="PSUM"))
        psum_t = ctx.enter_context(tc.tile_pool(name="psum_t", bufs=2, space="PSUM"))

        # --- inputs to SBUF ---
        # bpt chunks on the sync HWDGE ring; the rest on the scalar ring so
        # issue overlaps. Tile deps are view-range based, so the first
        # extract (j=0, c=0) starts as soon as the c=0 slice lands.
        bpt_all = consts.tile([P, W4], mybir.dt.uint16, name="bpt_all")
        xt_s = consts.tile([P, 32 * TOKENS], mybir.dt.bfloat16, name="xt_s")

        nc.scalar.dma_start(xt_s[:], xt[:, :])
        for c in range(4):
            nc.sync.dma_start(bpt_all[:, c * OPC:(c + 1) * OPC],
                              bpt[c * P:(c + 1) * P, :])

        scale_s = consts.tile([P, KCH], mybir.dt.float32, name="scale_s")
        nc.scalar.dma_start(scale_s[:], scale_t[:, :])

        # per-partition bias: rows 0:64 = -sum(x) per token, rows 64:128 = 0
        negsx_s = consts.tile([P, 1], mybir.dt.float32, name="negsx_s")
        nc.scalar.dma_start(negsx_s[:], negsx[:, :])

        # bias constant for the ScalarE Sign-plane (j=0): sign(v - 127.5)
        bias128 = consts.tile([P, 1], mybir.dt.float32, name="bias128")
        nc.vector.memset(bias128[:], -127.5)

        # M2: [128, 64] = [identity_64; identity_64] — the epilogue matmul
        # ybuf_chunk.T @ M2 transposes y AND sums the even/odd psum halves.
        m2 = consts.tile([P, TOKENS], mybir.dt.float16, name="m2")
        make_identity(nc, m2[:TOKENS, :])
        make_identity(nc, m2[TOKENS:, :])

        ybuf = out_pool.tile([P, OPC], mybir.dt.float16, name="ybuf")
        out_s = out_pool.tile([P, KCH * TOKENS], mybir.dt.float32, name="out_s")

        # even-c chains accumulate in rows 0:64 of pe tiles, odd-c chains
        # in rows 64:128 of po tiles (separate banks so each bank sees one
        # accumulation group; column tiling runs the two streams
        # concurrently in the PE array).
        pe_tiles = [
            psum_y.tile([P, w], mybir.dt.float32, name=f"psum_e{og}")
            for og, w in enumerate(OG_SIZES)
        ]
        po_tiles = [
            psum_y.tile([P, w], mybir.dt.float32, name=f"psum_o{og}")
            for og, w in enumerate(OG_SIZES)
        ]

        def plane_mm(plane_ap, j, c, og, col0):
            """col0: column in plane_ap where byte-chunk c starts."""
            m = j * 4 + c
            half = c % 2
            base = half * TOKENS
            tiles = po_tiles if half else pe_tiles
            w = OG_SIZES[og]
            s0 = col0 + OG_STARTS[og]
            nc.tensor.matmul(
                tiles[og][base:base + TOKENS, :],
                xt_s[:, m * TOKENS:(m + 1) * TOKENS],
                plane_ap[:, s0:s0 + w],
                start=(j == 0 and c == half),
                stop=(j == 7 and c == 2 + half),
                tile_position=(0, base),
            )

        def epilogue_og(og):
            w = OG_SIZES[og]
            s0, s1 = OG_STARTS[og], OG_STARTS[og] + w
            # PSUM -> SBUF with -sum(x)/0 per-row bias; even half on DVE,
            # odd half on ScalarE so they run in parallel
            nc.vector.tensor_scalar(
                ybuf[:TOKENS, s0:s1], pe_tiles[og][:TOKENS, :],
                negsx_s[:TOKENS, :], None, mybir.AluOpType.add,
            )
            nc.scalar.activation(
                ybuf[TOKENS:, s0:s1], po_tiles[og][TOKENS:, :],
                mybir.ActivationFunctionType.Identity,
                bias=negsx_s[TOKENS:, :], scale=1.0,
            )
            ks = list(OG_KS[og])
            pairs = [ks[i:i + 2] for i in range(0, len(ks), 2)]
            for pair in pairs:
                # [128,128].T @ [128,64] per chunk: transpose to [o, t] and
                # sum the even/odd token halves via stacked identities; two
                # chunks share one PSUM tile so one DVE op scales both
                pt = psum_t.tile([P, 2 * TOKENS], mybir.dt.float32,
                                 name="psum_t")
                for i, k in enumerate(pair):
                    nc.tensor.matmul(
                        pt[:, i * TOKENS:(i + 1) * TOKENS],
                        ybuf[:, k * P:(k + 1) * P], m2[:, :],
                        start=True, stop=True,
                    )
                k0, n = pair[0], len(pair)
                # per-output-row scale while copying PSUM -> SBUF (DVE is
                # idle during the epilogue; ScalarE still has copies)
                nc.vector.tensor_tensor(
                    out_s[:, k0 * TOKENS:(k0 + n) * TOKENS].rearrange(
                        "p (n t) -> p n t", n=n),
                    pt[:, :n * TOKENS].rearrange("p (n t) -> p n t", n=n),
                    scale_s[:, k0:k0 + n, None].to_broadcast((P, n, TOKENS)),
                    mybir.AluOpType.mult,
                )

        # --- unpack + matmul rounds ---
        # Sweep 1: all 8 bits on the c=0 quarter only — it lands first, so
        # the DVE works while the c1..3 DMA chunks are still in flight.
        # Sweep 2: bits over c1..3 at 3/4 width; j=7 og-major so each og's
        # epilogue overlaps the remaining matmuls.
        def extract_planes(j, src_ap, width, plane_q, uq, cast_engs):
            """Write bit-plane j of src_ap [P, width]; returns bf16 AP."""
            if j == 0:
                # MSB as a +-1 plane entirely on ScalarE: sign(v - 127.5).
                # Host uses 1*x weights for these features and excludes
                # them from the -sum(x) correction (as for j=7). Quarter
                # ops so each chunk's matmuls unlock as soon as possible.
                for q in range(width // OPC):
                    s2 = slice(q * OPC, (q + 1) * OPC)
                    nc.scalar.activation(
                        plane_q[:, s2], src_ap[:, s2],
                        mybir.ActivationFunctionType.Sign,
                        bias=bias128[:, :], scale=1.0,
                    )
                return plane_q
            if j == 7:
                # LSB plane as +-1 bf16 bit patterns in ONE uint16 op:
                # (v << 15) ^ 0xBF80 -> 0x3F80 (+1.0) if bit0 else 0xBF80
                # (-1.0). Host uses 1*x weights for these features and
                # excludes them from the -sum(x) correction.
                nc.vector.tensor_scalar(
                    uq[:, :width], src_ap, 15, 0xBF80,
                    mybir.AluOpType.logical_shift_left,
                    mybir.AluOpType.bitwise_xor,
                )
                return uq[:].bitcast(mybir.dt.bfloat16)
            # split wide extracts so downstream casts/matmuls start sooner
            step = 2 * OPC if width > 2 * OPC else width
            for at0 in range(0, width, step):
                s1 = slice(at0, min(at0 + step, width))
                nc.vector.tensor_scalar(
                    uq[:, s1], src_ap[:, s1], 7 - j, 1,
                    mybir.AluOpType.logical_shift_right,
                    mybir.AluOpType.bitwise_and,
                )
            # dtype-cast copies (bitwise TSP ops cannot cast); consecutive
            # same-engine quarters merge into one wider op
            n = width // OPC
            per = [n // len(cast_engs)] * len(cast_engs)
            per[0] += n - sum(per)
            merged = []
            for eng, k in zip(cast_engs, per):
                if merged and merged[-1][0] == eng:
                    merged[-1][1] += k
                else:
                    merged.append([eng, k])
            at = 0
            for eng, k in merged:
                s2 = slice(at, at + k * OPC)
                if eng == "dve":
                    nc.vector.tensor_copy(plane_q[:, s2], uq[:, s2])
                else:
                    nc.scalar.copy(plane_q[:, s2], uq[:, s2])
                at = s2.stop
            return plane_q

        for j in range(8):
            pq = plane_pool.tile([P, OPC], mybir.dt.bfloat16, name="pq1")
            u1 = upool.tile([P, OPC], mybir.dt.uint16, name="u1")
            pl = extract_planes(j, bpt_all[:, :OPC], OPC, pq, u1, ["dve"])
            for og in range(3):
                plane_mm(pl, j, 0, og, 0)

        W3 = 3 * OPC
        for j in range(8):
            pq = plane_pool.tile([P, W3], mybir.dt.bfloat16, name="pq3")
            u3 = upool.tile([P, W3], mybir.dt.uint16, name="u3")
            # ACT takes the last cast quarter; on j in {2, 4} also the 2nd
            engs = (["dve", "act", "act"] if j in (2, 4)
                    else ["dve", "dve", "act"])
            pl = extract_planes(j, bpt_all[:, OPC:], W3, pq, u3, engs)
            if j < 7:
                for c in range(1, 4):
                    for og in range(3):
                        plane_mm(pl, j, c, og, (c - 1) * OPC)
            else:
                for og in range(3):
                    for c in range(1, 4):
                        plane_mm(pl, j, c, og, (c - 1) * OPC)
                    epilogue_og(og)

        # output DMA chunked per og so early chunks overlap the remaining
        # epilogue work
        nc.sync.dma_start(yt[:, :4 * TOKENS], out_s[:, :4 * TOKENS])
        nc.sync.dma_start(yt[:, 4 * TOKENS:8 * TOKENS],
                          out_s[:, 4 * TOKENS:8 * TOKENS])
        nc.sync.dma_start(yt[:, 8 * TOKENS:], out_s[:, 8 * TOKENS:])

    nc.compile()
    return nc


def _prep_inputs(x, bp, scale):
    """Host-side re-layout of the full inputs into 8 per-core input maps."""
    x = np.asarray(x, dtype=np.float32)
    bp = np.asarray(bp)
    scale = np.asarray(scale, dtype=np.float32)

    # packed bytes, transposed to [g, o_padded]
    bpm = np.zeros((G, OUT_PAD), dtype=np.uint16)
    bpm[:, :OUT] = bp.astype(np.uint16).reshape(OUT, G).T

    # xt = 2 * x.T with rows permuted to (j, g) order, then split into
    # 128-row blocks laid out along the free dim: xt_dev[p, m*64 + t].
    # The j=7 (LSB) planes are +-1-valued, so those rows use 1*x and the
    # j=7 features are excluded from the -sum(x) correction.
    xT2 = (2.0 * x).T.astype(np.float32)            # [IN, TOKENS]
    perm = xT2.reshape(G, 8, TOKENS).transpose(1, 0, 2)  # [8, G, TOKENS]
    perm = perm.copy()
    perm[0] *= 0.5                                  # j=0 rows: 1*x (+-1 plane)
    perm[7] *= 0.5                                  # j=7 rows: 1*x (+-1 plane)
    perm = perm.reshape(IN, TOKENS)
    xt_dev = np.ascontiguousarray(
        perm.reshape(32, P, TOKENS).transpose(1, 0, 2).reshape(P, 32 * TOKENS)
    ).astype(ml_dtypes.bfloat16)

    negsx = np.zeros((P, 1), dtype=np.float32)
    x64 = x.astype(np.float64)
    negsx[:TOKENS, 0] = (
        -x64.sum(axis=1) + x64[:, 0::8].sum(axis=1) + x64[:, 7::8].sum(axis=1)
    ).astype(np.float32)

    scale_pad = np.zeros(OUT_PAD, dtype=np.float32)
    scale_pad[:OUT] = scale.reshape(-1)

    in_maps = []
    for cid in range(NCORES):
        sl = slice(cid * OPC, (cid + 1) * OPC)
        in_maps.append({
            "bpt": np.ascontiguousarray(bpm[:, sl]),
            "xt": xt_dev,
            "negsx": negsx,
            "scale_t": np.ascontiguousarray(
                scale_pad[sl].reshape(KCH, P).T),
        })
    return in_maps


def _assemble(results):
    """results: per-core {'yt': [128, 11*64]} -> full [64, OUT] fp32."""
    parts = []
    for cid in range(NCORES):
        a = np.asarray(results[cid]["yt"], dtype=np.float32)
        parts.append(a.reshape(P, KCH, TOKENS).transpose(1, 0, 2).reshape(OPC, TOKENS))
    full = np.concatenate(parts, axis=0)[:OUT]      # [OUT, TOKENS]
    return np.ascontiguousarray(full.T)             # [TOKENS, OUT]


def kernel(x, bp, scale, _trace=False):
    from concourse import bass_utils

    if "nc" not in _CACHE:
        _CACHE["nc"] = _build_bass()
    nc = _CACHE["nc"]

    in_maps = _prep_inputs(x, bp, scale)
    res = bass_utils.run_bass_kernel_spmd(
        nc, in_maps, core_ids=list(range(NCORES)), trace=_trace,
    )
    _CACHE["last_result"] = res
    return _assemble(res.results)
